# revision 1
# baseline (speedup 1.0000x reference)
"""Multi-head attention (B=2, T=2048, d_model=1024, H=16, hd=64) on 8 Trainium2
NeuronCores.

Sharding: the 32 (batch, head) attention units are split as 4 consecutive heads
of one batch per core (core c -> batch c//4, heads 4*(c%4) .. 4*(c%4)+3). Each
core computes its own QKV projection slice, causal attention for its heads, and
a partial out-projection (its 256 rows of W_out). The host sums the 4 partials
per batch and adds b_out.

Device-side layout (everything flows transposed so no on-chip transposes are
needed until the attention output):
  qT/kT [hd, T]  <- lhsT=W slice, rhs=xT
  v     [T, hd]  (+ ones column for the row-sum trick)
  sT    [k, q]   <- lhsT=kT chunk, rhs=qT          (psum, fp32)
  E     [k, q]   <- exp(sT * 1/sqrt(hd)) on ScalarE (bf16)
  pv    [q, hd+1]<- lhsT=E chunk, rhs=[v|1]        (col hd = row sum)
  a     [q, hd]  = pv[:, :hd] * (1/pv[:, hd])      (per-partition scalar)
  aT    [hd, T]  via DRAM round-trip DMA transpose
  out  += aT.T @ W_out slice                        (partial, fp32)
"""

import math
import os
from contextlib import ExitStack
from dataclasses import dataclass

import numpy as np
import ml_dtypes

import concourse.bass as bass
import concourse.tile as tile
from concourse import bacc, mybir
from concourse import bass_utils

AF = mybir.ActivationFunctionType
ALU = mybir.AluOpType
DT = mybir.dt

N_CORES = 8
NEG = -1e9


@dataclass(frozen=True)
class Cfg:
    T: int = 2048        # sequence length
    DM: int = 1024       # d_model
    HD: int = 64         # head dim
    NH: int = 4          # heads per core
    mode: str = "causal"  # "causal" | "full" | "bias"
    mm: str = "bf16"     # matmul operand dtype: "bf16" | "f32r" | "f32"

    @property
    def NHD(self):
        return self.NH * self.HD          # qkv slice width per core

    @property
    def KC(self):
        return self.DM // 128             # contraction chunks for projections

    @property
    def MC(self):
        return self.NHD // 128            # qT/kT partition chunks

    @property
    def TC(self):
        return self.T // 128              # t chunks

    @property
    def QW(self):
        return min(512, self.T)           # q group width

    @property
    def QG(self):
        return self.T // self.QW

    @property
    def QT(self):
        return self.QW // 128             # q tiles per group

    @property
    def EB(self):
        return self.DM // 512             # out-proj free blocks

    @property
    def mmdt(self):
        return {"bf16": DT.bfloat16, "f32r": DT.float32r, "f32": DT.float32}[self.mm]

    @property
    def npmm(self):
        return ml_dtypes.bfloat16 if self.mm == "bf16" else np.float32


def build_program(cfg: Cfg):
    """Build + compile the SPMD single-core program. Returns (nc, input_names)."""
    c = cfg
    assert c.DM % 128 == 0 and c.NHD % 128 == 0 and c.T % 512 == 0
    nc = bacc.Bacc("TRN2", target_bir_lowering=False, debug=False,
                   num_devices=N_CORES)
    f32 = DT.float32
    bf16 = DT.bfloat16
    mmdt = c.mmdt

    xT = nc.dram_tensor("xT", [c.DM, c.T], mmdt, kind="ExternalInput").ap()
    wq = nc.dram_tensor("wq", [c.DM, c.NHD], mmdt, kind="ExternalInput").ap()
    wk = nc.dram_tensor("wk", [c.DM, c.NHD], mmdt, kind="ExternalInput").ap()
    wv = nc.dram_tensor("wv", [c.DM, c.NHD], mmdt, kind="ExternalInput").ap()
    bq = nc.dram_tensor("bq", [128, c.MC], f32, kind="ExternalInput").ap()
    bk = nc.dram_tensor("bk", [128, c.MC], f32, kind="ExternalInput").ap()
    bvb = nc.dram_tensor("bvb", [128, c.NHD], f32, kind="ExternalInput").ap()
    wo = nc.dram_tensor("wo", [c.NHD, c.DM], mmdt, kind="ExternalInput").ap()
    maskb = None
    if c.mode == "bias":
        # additive bias, transposed: maskb[k, q]
        maskb = nc.dram_tensor("maskb", [c.T, c.T], f32, kind="ExternalInput").ap()
    out = nc.dram_tensor("out", [c.T, c.DM], f32, kind="ExternalOutput").ap()

    with tile.TileContext(nc) as tc, ExitStack() as ctx:
        _body(ctx, tc, c, xT, wq, wk, wv, bq, bk, bvb, wo, maskb, out)
    nc.compile()
    names = ["xT", "wq", "wk", "wv", "bq", "bk", "bvb", "wo"]
    if c.mode == "bias":
        names.append("maskb")
    return nc, names


def _body(ctx, tc, c: Cfg, xT, wq, wk, wv, bq, bk, bvb, wo, maskb, out):
    nc = tc.nc
    f32 = DT.float32
    bf16 = DT.bfloat16
    mmdt = c.mmdt
    causal = c.mode == "causal"
    scale = 1.0 / math.sqrt(c.HD)

    const = ctx.enter_context(tc.tile_pool(name="const", bufs=1))
    big = ctx.enter_context(tc.tile_pool(name="big", bufs=1))
    epool = ctx.enter_context(tc.tile_pool(name="E", bufs=c.TC))
    rpool = ctx.enter_context(tc.tile_pool(name="r", bufs=8))
    # PSUM: 3 x [128,1024] (6 banks) + 2 x [128,65] (2 banks) = 8 banks
    ps_mm = ctx.enter_context(tc.tile_pool(name="psmm", bufs=3, space="PSUM"))
    ps_pv = ctx.enter_context(tc.tile_pool(name="pspv", bufs=2, space="PSUM"))
    dramp = ctx.enter_context(tc.tile_pool(name="dram", bufs=1, space="DRAM"))
    bias_pool = None
    if c.mode == "bias":
        bias_pool = ctx.enter_context(tc.tile_pool(name="maskb", bufs=4))

    # ---- load inputs to SBUF ----
    bq_sb = const.tile([128, c.MC], f32, tag="bq")
    nc.sync.dma_start(out=bq_sb[:], in_=bq)
    bk_sb = const.tile([128, c.MC], f32, tag="bk")
    nc.sync.dma_start(out=bk_sb[:], in_=bk)
    bvb_sb = const.tile([128, c.NHD], f32, tag="bvb")
    nc.sync.dma_start(out=bvb_sb[:], in_=bvb)

    # consolidated input DMAs (one 3D-AP transfer each) — per-dma descriptor
    # generation on the sync sequencer is ~0.6us, so fewer, bigger DMAs
    # split along t so the first QKV block (which contracts over ALL chunks)
    # can start after the first half arrives
    xT_sb = big.tile([128, c.KC, c.T], mmdt, tag="xT")
    xTd = xT.rearrange("(c p) t -> p c t", p=128)
    TH = max(512, c.T // 2)
    for h in range(c.T // TH):
        nc.sync.dma_start(out=xT_sb[:, :, h * TH:(h + 1) * TH],
                          in_=xTd[:, :, h * TH:(h + 1) * TH])

    w_sbs = []
    for nm, w in (("wq", wq), ("wk", wk), ("wv", wv)):
        w_sb = big.tile([128, c.KC, c.NHD], mmdt, tag=nm)
        nc.sync.dma_start(out=w_sb[:],
                          in_=w.rearrange("(c p) n -> p c n", p=128))
        w_sbs.append(w_sb)
    wq_sb, wk_sb, wv_sb = w_sbs

    wo_sb = big.tile([128, c.MC, c.DM], mmdt, tag="wo")
    nc.sync.dma_start(out=wo_sb[:],
                      in_=wo.rearrange("(c p) n -> p c n", p=128))

    # causal mask block for diagonal tiles: tri[k, j] = 0 if j >= k else NEG
    tri = const.tile([128, 128], f32, tag="tri")
    nc.gpsimd.memset(tri[:], 0.0)
    nc.gpsimd.affine_select(
        out=tri[:], in_=tri[:],
        compare_op=ALU.is_ge, fill=NEG,
        base=0, channel_multiplier=-1, pattern=[[1, 128]],
    )

    # ---- QKV projections ----
    # psum tiles are [128, 1024] (2 banks); two 512-wide matmul groups per
    # tile, one wide DVE biased copy out.
    # qT is stored zero-padded per head ([128, NH, T], head h in partitions
    # (h%2)*64..+64, zeros elsewhere) so score matmuls can run with full
    # K=128 contraction: the other head's kT rows hit zeros. Full-K matmuls
    # keep the PE activity monitor busy -> 2.4 GHz instead of 1.2.
    qT_z = big.tile([128, c.NH, c.T], mmdt, tag="qT")
    nc.vector.memset(qT_z[:], 0.0)
    kT_sb = big.tile([128, c.MC, c.T], mmdt, tag="kT")
    HD1 = c.HD + 1
    v_sb = big.tile([128, c.TC, c.NH, HD1], bf16, tag="v")
    nc.vector.memset(v_sb[:, :, :, c.HD:HD1], 1.0)
    W2 = min(1024, c.T)
    VG = min(c.TC, max(1, 1024 // c.NHD))    # t-chunks per v psum tile

    def emit_qk_tile(m, w_sb, b_sb, which, n):
        ps = ps_mm.tile([128, 1024], f32, tag="mm")
        for d in range(W2 // 512):
            for k in range(c.KC):
                nc.tensor.matmul(
                    ps[:, d * 512:(d + 1) * 512],
                    lhsT=w_sb[:, k, m * 128:(m + 1) * 128],
                    rhs=xT_sb[:, k, n * W2 + d * 512:n * W2 + (d + 1) * 512],
                    start=(k == 0), stop=(k == c.KC - 1),
                )
        sl = slice(n * W2, (n + 1) * W2)
        if which == "k":
            nc.vector.tensor_scalar_add(
                kT_sb[:, m, sl], ps[:, 0:W2], b_sb[:, m:m + 1],
            )
        else:
            nc.vector.tensor_scalar_add(
                qT_z[0:64, 2 * m, sl], ps[0:64, 0:W2], b_sb[0:64, m:m + 1],
            )
            nc.vector.tensor_scalar_add(
                qT_z[64:128, 2 * m + 1, sl], ps[64:128, 0:W2],
                b_sb[64:128, m:m + 1],
            )

    def emit_qk(m):
        for w_sb, b_sb, which in ((wq_sb, bq_sb, "q"), (wk_sb, bk_sb, "k")):
            for n in range(c.T // W2):
                emit_qk_tile(m, w_sb, b_sb, which, n)

    def emit_v_tile(tg):
        # v in normal layout, augmented with a ones column per head;
        # VG t-chunks share one psum tile.
        ps = ps_mm.tile([128, 1024], f32, tag="mm")
        for d in range(VG):
            t = tg * VG + d
            for k in range(c.KC):
                nc.tensor.matmul(
                    ps[:, d * c.NHD:(d + 1) * c.NHD],
                    lhsT=xT_sb[:, k, t * 128:(t + 1) * 128],
                    rhs=wv_sb[:, k, :],
                    start=(k == 0), stop=(k == c.KC - 1),
                )
        for d in range(VG):
            t = tg * VG + d
            nc.vector.tensor_tensor(
                out=v_sb[:, t, :, 0:c.HD],
                in0=ps[:, d * c.NHD:(d + 1) * c.NHD].rearrange(
                    "p (h d) -> p h d", d=c.HD),
                in1=bvb_sb.rearrange("p (h d) -> p h d", d=c.HD),
                op=ALU.add,
            )

    # (emission of qk/v/attention is interleaved below: head-pair hp's
    # attention is emitted before chunk hp+1's q/k so the psum-slot FIFO
    # doesn't serialize attention behind the whole projection phase)

    # ---- attention (head-pair outer, q-group inner) ----
    # kc chunks are paired into [128, 1024] psum tiles so one exp covers
    # 1024 columns. Scores run with full K=128 contraction against the
    # natural two-head kT chunk (zero-padded qT kills the other head's
    # contribution), which keeps the PE activity monitor at 2.4 GHz.
    a_sb = big.tile([128, c.TC, c.NH, c.HD], bf16, tag="a")
    a_dram = dramp.tile([c.T, c.NHD], bf16, tag="adram")
    aT_sb = big.tile([128, c.MC, c.T], bf16, tag="aT")
    ostage = ctx.enter_context(tc.tile_pool(name="ostage", bufs=4))

    def attn_hp(hp, fillers=()):
        fillers = list(fillers)
        per_g = -(-len(fillers) // c.QG) if fillers else 0
        for g in range(c.QG):
            for _ in range(per_g):
                if fillers:
                    fillers.pop(0)()
            kmax = (g + 1) * c.QT if causal else c.TC
            assert kmax % 2 == 0
            etiles = {}                      # (hl, kp) -> [128, 1024] E tile
            for kp in range(kmax // 2):
                for hl in range(2):
                    h = 2 * hp + hl
                    ps = ps_mm.tile([128, 1024], f32, tag="mm")
                    for d in range(2):
                        kc = 2 * kp + d
                        nc.tensor.matmul(
                            ps[:, d * 512:d * 512 + c.QW],
                            lhsT=kT_sb[:, hp, kc * 128:(kc + 1) * 128],
                            rhs=qT_z[:, h, g * c.QW:(g + 1) * c.QW],
                            start=True, stop=True,
                        )
                        if causal:
                            off = (kc - g * c.QT) * 128
                            if off >= 0:
                                nc.vector.tensor_tensor(
                                    out=ps[:, d * 512 + off:d * 512 + off + 128],
                                    in0=ps[:, d * 512 + off:d * 512 + off + 128],
                                    in1=tri[:], op=ALU.add,
                                )
                        elif c.mode == "bias":
                            mb = bias_pool.tile([128, c.QW], f32, tag="mb")
                            nc.sync.dma_start(
                                out=mb[:],
                                in_=maskb[kc * 128:(kc + 1) * 128,
                                          g * c.QW:(g + 1) * c.QW],
                            )
                            nc.vector.tensor_tensor(
                                out=ps[:, d * 512:d * 512 + c.QW],
                                in0=ps[:, d * 512:d * 512 + c.QW],
                                in1=mb[:], op=ALU.add,
                            )
                    et = epool.tile([128, 1024], bf16, tag="E")
                    nc.scalar.activation(et[:], ps[:], AF.Exp, scale=scale)
                    etiles[(hl, kp)] = et
            for hl in range(2):
                h = 2 * hp + hl
                for j in range(c.QT):
                    qt = g * c.QT + j
                    kn = qt + 1 if causal else c.TC
                    psv = ps_pv.tile([128, HD1], f32, tag="pv")
                    for kc in range(kn):
                        kp, d = divmod(kc, 2)
                        nc.tensor.matmul(
                            psv[:],
                            lhsT=etiles[(hl, kp)][
                                :, d * 512 + j * 128:d * 512 + (j + 1) * 128],
                            rhs=v_sb[:, kc, h, :],
                            start=(kc == 0), stop=(kc == kn - 1),
                        )
                    r = rpool.tile([128, 1], f32, tag="r")
                    nc.vector.reciprocal(r[:], psv[:, c.HD:HD1])
                    nc.vector.tensor_scalar_mul(
                        a_sb[:, qt, h, :], psv[:, 0:c.HD], r[:, 0:1],
                    )
                    # stream a out to DRAM as soon as each t-chunk is done
                    if hp == c.NH // 2 - 1 and hl == 1:
                        nc.sync.dma_start(
                            out=a_dram[qt * 128:(qt + 1) * 128, :],
                            in_=a_sb[:, qt, :, :],
                        )

            # transpose this group's a rows -> aT as soon as they're final
            if hp == c.NH // 2 - 1:
                for ci in range(c.MC):
                    nc.sync.dma_start(
                        out=aT_sb[:, ci, g * c.QW:(g + 1) * c.QW],
                        in_=a_dram[g * c.QW:(g + 1) * c.QW,
                                   ci * 128:(ci + 1) * 128],
                        transpose=True,
                    )

    # Head pair hp needs only q/k chunk hp (+v). Emit pair hp+1's projection
    # tiles as fillers inside pair hp's attention groups so they overlap the
    # ACT-bound exp phase instead of serializing behind it in the psum FIFO.
    emit_qk(0)
    for tg in range(c.TC // VG):
        emit_v_tile(tg)
    for m in range(1, c.NH // 2):
        emit_qk(m)
    for hp in range(c.NH // 2):
        attn_hp(hp)

    # ---- partial out-projection ----
    EW = min(1024, c.DM)
    for t in range(c.TC):
        for ebg in range(c.DM // EW):
            ps = ps_mm.tile([128, 1024], f32, tag="mm")
            for d in range(EW // 512):
                e0 = ebg * EW + d * 512
                for ci in range(c.MC):
                    nc.tensor.matmul(
                        ps[:, d * 512:(d + 1) * 512],
                        lhsT=aT_sb[:, ci, t * 128:(t + 1) * 128],
                        rhs=wo_sb[:, ci, e0:e0 + 512],
                        start=(ci == 0), stop=(ci == c.MC - 1),
                    )
            ot = ostage.tile([128, EW], f32, tag="o")
            if t % 2 == 0:
                nc.vector.tensor_copy(ot[:], ps[:, 0:EW])
            else:
                nc.scalar.copy(ot[:], ps[:, 0:EW])
            nc.sync.dma_start(
                out=out[t * 128:(t + 1) * 128, ebg * EW:(ebg + 1) * EW],
                in_=ot[:],
            )


# ---------------------------------------------------------------------------
# host side
# ---------------------------------------------------------------------------

_CACHE: dict = {}


def _get_program(cfg: Cfg):
    key = cfg
    if key not in _CACHE:
        _CACHE[key] = build_program(cfg)
    return _CACHE[key]


def _mask_mode(mask: np.ndarray, T: int) -> str:
    m = (np.asarray(mask).reshape(T, T) != 0)
    if m.all():
        return "full"
    if np.array_equal(m, np.tril(np.ones((T, T), dtype=bool))):
        return "causal"
    return "bias"


def make_in_maps(cfg: Cfg, x, W_qkv, b_qkv, W_out, mask=None):
    """Slice full inputs into the 8 per-core input dicts."""
    c = cfg
    npmm = c.npmm
    B = x.shape[0]
    n_hg = N_CORES // B                      # head groups per batch
    in_maps = []
    maskb = None
    if c.mode == "bias":
        m = (np.asarray(mask).reshape(c.T, c.T) != 0)
        maskb = np.where(m, np.float32(0), np.float32(NEG)).T.copy()
    for core in range(N_CORES):
        b, hg = divmod(core, n_hg)
        col0 = hg * c.NHD
        xT = np.ascontiguousarray(x[b].T).astype(npmm)
        wq_ = np.ascontiguousarray(W_qkv[:, 0 * c.DM + col0:0 * c.DM + col0 + c.NHD]).astype(npmm)
        wk_ = np.ascontiguousarray(W_qkv[:, 1 * c.DM + col0:1 * c.DM + col0 + c.NHD]).astype(npmm)
        wv_ = np.ascontiguousarray(W_qkv[:, 2 * c.DM + col0:2 * c.DM + col0 + c.NHD]).astype(npmm)
        bq_ = np.ascontiguousarray(
            b_qkv[0 * c.DM + col0:0 * c.DM + col0 + c.NHD].reshape(c.MC, 128).T
        ).astype(np.float32)
        bk_ = np.ascontiguousarray(
            b_qkv[1 * c.DM + col0:1 * c.DM + col0 + c.NHD].reshape(c.MC, 128).T
        ).astype(np.float32)
        bv_ = b_qkv[2 * c.DM + col0:2 * c.DM + col0 + c.NHD].astype(np.float32)
        bvb_ = np.ascontiguousarray(np.broadcast_to(bv_, (128, c.NHD)))
        wo_ = np.ascontiguousarray(W_out[col0:col0 + c.NHD, :]).astype(npmm)
        im = dict(xT=xT, wq=wq_, wk=wk_, wv=wv_, bq=bq_, bk=bk_, bvb=bvb_,
                  wo=wo_)
        if c.mode == "bias":
            im["maskb"] = maskb
        in_maps.append(im)
    return in_maps


def run_sharded(cfg: Cfg, x, W_qkv, b_qkv, W_out, b_out, mask=None, **kw):
    """Run the SPMD program on 8 cores and assemble the full output."""
    nc, _names = _get_program(cfg)
    in_maps = make_in_maps(cfg, x, W_qkv, b_qkv, W_out, mask)
    res = bass_utils.run_bass_kernel_spmd(
        nc, in_maps, core_ids=list(range(N_CORES)), **kw,
    )
    outs = [r["out"] for r in res.results]
    B = x.shape[0]
    n_hg = N_CORES // B
    y = np.stack([
        np.sum(outs[b * n_hg:(b + 1) * n_hg], axis=0) for b in range(B)
    ]) + b_out.astype(np.float32)
    return y.astype(np.float32), res


def kernel(x, W_qkv, b_qkv, W_out, b_out, mask):
    x = np.asarray(x, dtype=np.float32)
    W_qkv = np.asarray(W_qkv, dtype=np.float32)
    b_qkv = np.asarray(b_qkv, dtype=np.float32)
    W_out = np.asarray(W_out, dtype=np.float32)
    b_out = np.asarray(b_out, dtype=np.float32)
    B, T, DM = x.shape
    mode = _mask_mode(mask, T)
    cfg = Cfg(T=T, DM=DM, mode=mode, mm=os.environ.get("MHA_MM_DT", "bf16"))
    y, _ = run_sharded(cfg, x, W_qkv, b_qkv, W_out, b_out, mask)
    return y



# revision 4
# speedup vs baseline: 1.1522x; 1.1522x over previous
"""Multi-head attention (B=2, T=2048, d_model=1024, H=16, hd=64) on 8 Trainium2
NeuronCores.

Sharding: the 32 (batch, head) attention units are split as 4 consecutive heads
of one batch per core (core c -> batch c//4, heads 4*(c%4) .. 4*(c%4)+3). Each
core computes its own QKV projection slice, causal attention for its heads, and
a partial out-projection (its 256 rows of W_out). The host sums the 4 partials
per batch and adds the output bias.

Device-side layout (everything flows transposed so no on-chip transposes are
needed until the attention output):
  qT/kT [hd, T]  <- lhsT=W slice, rhs=xT   (per-head halves in partitions)
  v     [T, hd]  (+ ones column for the row-sum trick)
  sT    [k, q]   <- K=64 row-tiled matmul PAIRS: head 2m in PE rows 0-63,
                    head 2m+1 in rows 64-127, issued back-to-back so the two
                    heads' score matmuls run CONCURRENTLY in disjoint row
                    strips (~2x score throughput vs zero-padded full-K).
  E     [k, q]   <- exp(sT * 1/sqrt(hd)) on ScalarE over [128,<=1536] psum
                    tiles (3 k-chunks per ACTIVATE to amortize the ~350-cycle
                    per-instruction overhead); causal masking of diagonal
                    blocks done AFTER exp by zeroing on the (otherwise idle)
                    GPSIMD engine.
  pv    [q, 4, hd+1] <- lhsT=E chunk, rhs=[v|1]; 4 q-tile accumulation chains
                    share one PSUM bank; normalization is one reciprocal +
                    one broadcast multiply per (head, q-group).
  aT    [hd, T]  via DRAM round-trip DMA transpose (off the critical engines)
  out  += aT.T @ W_out slice  (partial, fp32)

Exact bias folds (no accuracy cost):
  - k bias: (q+bq)@bk is constant along the softmax axis -> dropped entirely.
  - v bias: attention weights sum to 1, so out = a_nb@W_out + bv@W_out; the
    constant row bv@W_out is added on the host together with b_out.

Scheduling: q/k/v projections for head-pair 1 and the v tiles are emitted as
fillers inside head-pair 0's attention groups so ScalarE (the near-critical
engine: ~90us of exp) starts within ~10us instead of after all projections.
Head-pair 0 walks q-groups ascending (earliest possible first exp), head-pair
1 descending with each group's out-projection emitted one group later, so the
kernel tail is the smallest group's out-projection only.
"""

import math
import os
from contextlib import ExitStack
from dataclasses import dataclass

import numpy as np
import ml_dtypes

import concourse.bass as bass
import concourse.tile as tile
from concourse import bacc, mybir
from concourse import bass_utils

AF = mybir.ActivationFunctionType
ALU = mybir.AluOpType
DT = mybir.dt

N_CORES = 8


@dataclass(frozen=True)
class Cfg:
    T: int = 2048        # sequence length
    DM: int = 1024       # d_model
    HD: int = 64         # head dim
    NH: int = 4          # heads per core
    mode: str = "causal"  # "causal" | "full"
    mm: str = "bf16"     # matmul operand dtype: "bf16" | "f32r" | "f32"

    @property
    def NHD(self):
        return self.NH * self.HD          # qkv slice width per core

    @property
    def KC(self):
        return self.DM // 128             # contraction chunks for projections

    @property
    def MC(self):
        return self.NHD // 128            # qT/kT partition chunks

    @property
    def TC(self):
        return self.T // 128              # t chunks

    @property
    def QW(self):
        return min(512, self.T)           # q group width

    @property
    def QG(self):
        return self.T // self.QW

    @property
    def QT(self):
        return self.QW // 128             # q tiles per group

    @property
    def mmdt(self):
        return {"bf16": DT.bfloat16, "f32r": DT.float32r, "f32": DT.float32}[self.mm]

    @property
    def npmm(self):
        return ml_dtypes.bfloat16 if self.mm == "bf16" else np.float32


def build_program(cfg: Cfg):
    """Build + compile the SPMD single-core program. Returns (nc, input_names)."""
    c = cfg
    assert c.DM % 128 == 0 and c.NHD % 128 == 0 and c.T % 512 == 0
    assert c.mode in ("causal", "full")
    nc = bacc.Bacc("TRN2", target_bir_lowering=False, debug=False,
                   num_devices=N_CORES)
    f32 = DT.float32
    mmdt = c.mmdt

    xT = nc.dram_tensor("xT", [c.DM, c.T], mmdt, kind="ExternalInput").ap()
    wq = nc.dram_tensor("wq", [c.DM, c.NHD], mmdt, kind="ExternalInput").ap()
    wk = nc.dram_tensor("wk", [c.DM, c.NHD], mmdt, kind="ExternalInput").ap()
    wv = nc.dram_tensor("wv", [c.DM, c.NHD], mmdt, kind="ExternalInput").ap()
    bq = nc.dram_tensor("bq", [128, c.MC], f32, kind="ExternalInput").ap()
    wo = nc.dram_tensor("wo", [c.NHD, c.DM], mmdt, kind="ExternalInput").ap()
    out = nc.dram_tensor("out", [c.T, c.DM], f32, kind="ExternalOutput").ap()

    with tile.TileContext(nc) as tc, ExitStack() as ctx:
        _body(ctx, tc, c, xT, wq, wk, wv, bq, wo, out)
    nc.compile()
    names = ["xT", "wq", "wk", "wv", "bq", "wo"]
    return nc, names


def _body(ctx, tc, c: Cfg, xT, wq, wk, wv, bq, wo, out):
    nc = tc.nc
    f32 = DT.float32
    bf16 = DT.bfloat16
    mmdt = c.mmdt
    causal = c.mode == "causal"
    scale = 1.0 / math.sqrt(c.HD)
    HD1 = c.HD + 1

    const = ctx.enter_context(tc.tile_pool(name="const", bufs=1))
    big = ctx.enter_context(tc.tile_pool(name="big", bufs=1))
    epool = ctx.enter_context(tc.tile_pool(name="E", bufs=14))
    rpool = ctx.enter_context(tc.tile_pool(name="r", bufs=4))
    # PSUM: 2 x [128,1536] (3 banks each) + 2 x [128,4*65] (1 bank each) = 8
    ps_mm = ctx.enter_context(tc.tile_pool(name="psmm", bufs=2, space="PSUM"))
    ps_pv = ctx.enter_context(tc.tile_pool(name="pspv", bufs=2, space="PSUM"))
    dramp = ctx.enter_context(tc.tile_pool(name="dram", bufs=1, space="DRAM"))
    ostage = ctx.enter_context(tc.tile_pool(name="ostage", bufs=4))

    # ---- input DMAs, ordered so the first projection matmuls start early ----
    bq_sb = const.tile([128, c.MC], f32, tag="bq")
    nc.sync.dma_start(out=bq_sb[:], in_=bq)

    wk_sb = big.tile([128, c.KC, c.NHD], mmdt, tag="wk")
    nc.sync.dma_start(out=wk_sb[:], in_=wk.rearrange("(c p) n -> p c n", p=128))
    wq_sb = big.tile([128, c.KC, c.NHD], mmdt, tag="wq")
    nc.sync.dma_start(out=wq_sb[:], in_=wq.rearrange("(c p) n -> p c n", p=128))

    xT_sb = big.tile([128, c.KC, c.T], mmdt, tag="xT")
    xTd = xT.rearrange("(c p) t -> p c t", p=128)
    TH = 512
    for h in range(c.T // TH):
        nc.sync.dma_start(out=xT_sb[:, :, h * TH:(h + 1) * TH],
                          in_=xTd[:, :, h * TH:(h + 1) * TH])

    wv_sb = big.tile([128, c.KC, c.NHD], mmdt, tag="wv")
    nc.sync.dma_start(out=wv_sb[:], in_=wv.rearrange("(c p) n -> p c n", p=128))
    wo_sb = big.tile([128, c.MC, c.DM], mmdt, tag="wo")
    nc.sync.dma_start(out=wo_sb[:],
                      in_=wo.rearrange("(c p) n -> p c n", p=128))

    # ---- persistent SBUF tensors ----
    qT_z = big.tile([128, c.NH, c.T], mmdt, tag="qT")
    kT_sb = big.tile([128, c.MC, c.T], mmdt, tag="kT")
    v_sb = big.tile([128, c.TC, c.NH, HD1], bf16, tag="v")
    nc.vector.memset(v_sb[:, :, :, c.HD:HD1], 1.0)
    a_sb = big.tile([128, c.TC, c.NH, c.HD], bf16, tag="a")
    a_dram = dramp.tile([c.T, c.NHD], bf16, tag="adram")
    aT_sb = big.tile([128, c.MC, c.T], bf16, tag="aT")

    W2 = 512                               # projection tile width
    VG = 4                                 # t-chunks per v psum tile

    # ---- projections (emitted piecewise; later pieces become fillers) ----
    def emit_k_tile(m, n):
        ps = ps_mm.tile([128, W2], f32, tag="mm")
        for k in range(c.KC):
            nc.tensor.matmul(
                ps[:],
                lhsT=wk_sb[:, k, m * 128:(m + 1) * 128],
                rhs=xT_sb[:, k, n * W2:(n + 1) * W2],
                start=(k == 0), stop=(k == c.KC - 1),
            )
        nc.vector.tensor_copy(kT_sb[:, m, n * W2:(n + 1) * W2], ps[:])

    def emit_q_tile(m, n):
        ps = ps_mm.tile([128, W2], f32, tag="mm")
        for k in range(c.KC):
            nc.tensor.matmul(
                ps[:],
                lhsT=wq_sb[:, k, m * 128:(m + 1) * 128],
                rhs=xT_sb[:, k, n * W2:(n + 1) * W2],
                start=(k == 0), stop=(k == c.KC - 1),
            )
        sl = slice(n * W2, (n + 1) * W2)
        nc.vector.tensor_scalar_add(
            qT_z[0:64, 2 * m, sl], ps[0:64, :], bq_sb[0:64, m:m + 1])
        nc.vector.tensor_scalar_add(
            qT_z[64:128, 2 * m + 1, sl], ps[64:128, :], bq_sb[64:128, m:m + 1])

    def emit_v_tile(tg):
        # v in normal layout (no bias: folded into the host-side output bias)
        ps = ps_mm.tile([128, VG * c.NHD], f32, tag="mm")
        for d in range(VG):
            t = tg * VG + d
            for k in range(c.KC):
                nc.tensor.matmul(
                    ps[:, d * c.NHD:(d + 1) * c.NHD],
                    lhsT=xT_sb[:, k, t * 128:(t + 1) * 128],
                    rhs=wv_sb[:, k, :],
                    start=(k == 0), stop=(k == c.KC - 1),
                )
        nc.vector.tensor_copy(
            v_sb[:, tg * VG:(tg + 1) * VG, :, 0:c.HD],
            ps[:].rearrange("p (d h e) -> p d h e", d=VG, e=c.HD),
        )

    # ---- out-projection for one q-group ----
    def emit_outproj(g):
        for t in range(g * c.QT, (g + 1) * c.QT):
            ps = ps_mm.tile([128, c.DM], f32, tag="mm")
            for d in range(c.DM // 512):
                for ci in range(c.MC):
                    nc.tensor.matmul(
                        ps[:, d * 512:(d + 1) * 512],
                        lhsT=aT_sb[:, ci, t * 128:(t + 1) * 128],
                        rhs=wo_sb[:, ci, d * 512:(d + 1) * 512],
                        start=(ci == 0), stop=(ci == c.MC - 1),
                    )
            ot = ostage.tile([128, c.DM], f32, tag="o")
            nc.vector.tensor_copy(ot[:], ps[:])
            nc.sync.dma_start(out=out[t * 128:(t + 1) * 128, :], in_=ot[:])

    # ---- attention for one head pair over one q-group ----
    def attn_group(hp, g, fillers):
        kmax = (g + 1) * c.QT if causal else c.TC
        # scores + exp in tiles of up to 3 k-chunks, both heads interleaved
        # (K=64 row-tiled pairs run concurrently in disjoint PE row strips)
        etiles = ([], [])                   # per hl: list of E tiles
        c0 = 0
        while c0 < kmax:
            nch = min(3, kmax - c0)
            pst = [ps_mm.tile([128, 1536], f32, tag="mm", name="ps_s")
                   for _ in range(2)]
            s0 = None
            for ci in range(nch):
                kc = c0 + ci
                jj = kc - g * c.QT if causal else -1
                off = jj * 128 if jj >= 0 else 0
                if ci == 0:
                    s0 = off
                for hl in range(2):
                    h = 2 * hp + hl
                    b0 = hl * 64
                    nc.tensor.matmul(
                        pst[hl][:, ci * 512 + off:(ci + 1) * 512],
                        lhsT=kT_sb[b0:b0 + 64, hp, kc * 128:(kc + 1) * 128],
                        rhs=qT_z[b0:b0 + 64, h, g * c.QW + off:(g + 1) * c.QW],
                        start=True, stop=True,
                    )
            for hl in range(2):
                et = epool.tile([128, 1536], bf16, tag="E")
                nc.scalar.activation(
                    et[:, s0:nch * 512], pst[hl][:, s0:nch * 512],
                    AF.Exp, scale=scale,
                )
                # zero the strictly-upper triangle of diagonal blocks (the
                # masked region) on the otherwise idle GPSIMD engine
                for ci in range(nch):
                    jj = c0 + ci - g * c.QT if causal else -1
                    if jj >= 0:
                        blk = et[:, ci * 512 + jj * 128:ci * 512 + (jj + 1) * 128]
                        nc.gpsimd.affine_select(
                            out=blk, in_=blk,
                            compare_op=ALU.is_ge, fill=0.0,
                            base=0, channel_multiplier=-1, pattern=[[1, 128]],
                        )
                etiles[hl].append(et)
            c0 += nch

        for f in fillers:
            f()

        # PV: 4 accumulation chains (one per q-tile) share one PSUM bank
        for hl in range(2):
            h = 2 * hp + hl
            pv = ps_pv.tile([128, c.QT, HD1], f32, tag="pv")
            for j in range(c.QT):
                qt = g * c.QT + j
                kn = qt + 1 if causal else c.TC
                for kc in range(kn):
                    ti, tci = divmod(kc, 3)
                    nc.tensor.matmul(
                        pv[:, j, :],
                        lhsT=etiles[hl][ti][
                            :, tci * 512 + j * 128:tci * 512 + (j + 1) * 128],
                        rhs=v_sb[:, kc, h, :],
                        start=(kc == 0), stop=(kc == kn - 1),
                    )
            r = rpool.tile([128, c.QT, 1], f32, tag="r")
            nc.vector.reciprocal(r[:, :, 0], pv[:, :, c.HD])
            nc.vector.tensor_tensor(
                out=a_sb[:, g * c.QT:(g + 1) * c.QT, h, :],
                in0=pv[:, :, 0:c.HD],
                in1=r.broadcast_to([128, c.QT, c.HD]),
                op=ALU.mult,
            )

    # transpose a -> aT for one q-group via DRAM round trip (DMA engines)
    def emit_transpose(g):
        nc.sync.dma_start(
            out=a_dram[g * c.QW:(g + 1) * c.QW, :].rearrange(
                "(q p) n -> p q n", p=128),
            in_=a_sb[:, g * c.QT:(g + 1) * c.QT, :, :],
        )
        for ci in range(c.MC):
            nc.sync.dma_start(
                out=aT_sb[:, ci, g * c.QW:(g + 1) * c.QW],
                in_=a_dram[g * c.QW:(g + 1) * c.QW, ci * 128:(ci + 1) * 128],
                transpose=True,
            )

    # ---- emission schedule ----
    # head-pair 0 projections first (k then q per n-block, tracking the xT
    # DMA arrival), plus the first v tile group; everything else becomes
    # fillers inside head-pair 0's attention groups.
    for n in range(c.T // W2):
        emit_k_tile(0, n)
        emit_q_tile(0, n)
    emit_v_tile(0)

    fillers = []
    for tg in range(1, c.TC // VG):
        fillers.append(lambda tg=tg: emit_v_tile(tg))
    for n in range(c.T // W2):
        fillers.append(lambda n=n: emit_k_tile(1, n))
    for n in range(c.T // W2):
        fillers.append(lambda n=n: emit_q_tile(1, n))

    # weight fillers by each group's exp backlog, keeping v tile tg
    # available no later than group tg (PV of group g reads v chunks <= g)
    groups0 = list(range(c.QG)) if causal else list(range(c.QG))
    share = [1, 2, 3, len(fillers)]  # cumulative pop targets per group index
    popped = 0
    for gi, g in enumerate(groups0):
        want = share[gi] if gi < len(share) else len(fillers)
        take = max(0, min(want, len(fillers)) - popped)
        fs, popped = fillers[popped:popped + take], popped + take
        attn_group(0, g, fs)
    # any leftovers (non-causal paths)
    rest, popped = fillers[popped:], len(fillers)

    # head-pair 1: descending groups; out-projection of the previous group
    # is emitted as a filler inside the next group so its transpose DMAs
    # overlap compute. The last group's out-projection is the kernel tail.
    groups1 = list(range(c.QG - 1, -1, -1))
    prev = None
    for g in groups1:
        fs = list(rest)
        rest = []
        if prev is not None:
            fs.append(lambda p=prev: emit_outproj(p))
        attn_group(1, g, fs)
        emit_transpose(g)
        prev = g
    emit_outproj(prev)


# ---------------------------------------------------------------------------
# host side
# ---------------------------------------------------------------------------

_CACHE: dict = {}


def _get_program(cfg: Cfg):
    key = cfg
    if key not in _CACHE:
        _CACHE[key] = build_program(cfg)
    return _CACHE[key]


def _mask_mode(mask: np.ndarray, T: int) -> str:
    m = (np.asarray(mask).reshape(T, T) != 0)
    if m.all():
        return "full"
    if np.array_equal(m, np.tril(np.ones((T, T), dtype=bool))):
        return "causal"
    raise NotImplementedError("only causal/full masks supported")


def make_in_maps(cfg: Cfg, x, W_qkv, b_qkv, W_out, mask=None):
    """Slice full inputs into the 8 per-core input dicts."""
    c = cfg
    npmm = c.npmm
    B = x.shape[0]
    n_hg = N_CORES // B                      # head groups per batch
    in_maps = []
    for core in range(N_CORES):
        b, hg = divmod(core, n_hg)
        col0 = hg * c.NHD
        xT_ = np.ascontiguousarray(x[b].T).astype(npmm)
        wq_ = np.ascontiguousarray(W_qkv[:, col0:col0 + c.NHD]).astype(npmm)
        wk_ = np.ascontiguousarray(
            W_qkv[:, c.DM + col0:c.DM + col0 + c.NHD]).astype(npmm)
        wv_ = np.ascontiguousarray(
            W_qkv[:, 2 * c.DM + col0:2 * c.DM + col0 + c.NHD]).astype(npmm)
        bq_ = np.ascontiguousarray(
            b_qkv[col0:col0 + c.NHD].reshape(c.MC, 128).T).astype(np.float32)
        wo_ = np.ascontiguousarray(W_out[col0:col0 + c.NHD, :]).astype(npmm)
        in_maps.append(dict(xT=xT_, wq=wq_, wk=wk_, wv=wv_, bq=bq_, wo=wo_))
    return in_maps


def run_sharded(cfg: Cfg, x, W_qkv, b_qkv, W_out, b_out, mask=None, **kw):
    """Run the SPMD program on 8 cores and assemble the full output."""
    nc, _names = _get_program(cfg)
    in_maps = make_in_maps(cfg, x, W_qkv, b_qkv, W_out, mask)
    res = bass_utils.run_bass_kernel_spmd(
        nc, in_maps, core_ids=list(range(N_CORES)), **kw,
    )
    outs = [np.asarray(r["out"], dtype=np.float32) for r in res.results]
    B = x.shape[0]
    n_hg = N_CORES // B
    # v bias folded here: attention weights sum to 1, so the v-bias term is
    # the constant row bv @ W_out
    bv = b_qkv[2 * cfg.DM:3 * cfg.DM].astype(np.float32)
    b_eff = b_out.astype(np.float32) + bv @ W_out.astype(np.float32)
    y = np.stack([
        np.sum(outs[b * n_hg:(b + 1) * n_hg], axis=0) for b in range(B)
    ]) + b_eff
    return y.astype(np.float32), res


def kernel(x, W_qkv, b_qkv, W_out, b_out, mask):
    x = np.asarray(x, dtype=np.float32)
    W_qkv = np.asarray(W_qkv, dtype=np.float32)
    b_qkv = np.asarray(b_qkv, dtype=np.float32)
    W_out = np.asarray(W_out, dtype=np.float32)
    b_out = np.asarray(b_out, dtype=np.float32)
    B, T, DM = x.shape
    mode = _mask_mode(mask, T)
    cfg = Cfg(T=T, DM=DM, mode=mode, mm=os.environ.get("MHA_MM_DT", "bf16"))
    y, _ = run_sharded(cfg, x, W_qkv, b_qkv, W_out, b_out, mask)
    return y
